# revision 18
# baseline (speedup 1.0000x reference)
"""LoRA row-parallel linear on 8 TRN2 NeuronCores.

Problem: y = x @ W^T + delta, where per-token LoRA delta[t] = B[s] @ (A[s] @ x[t]),
s = token_to_slot[t] (8 adapters, rank 16, scaling baked into B).

Strategy: token data-parallel across the 8 cores (T=8192 -> 1024 tokens/core),
no collectives; each core computes its token block in transposed output space
(y^T, un-transposed on the host).

Precision/speed (gate: max-rel < 2e-2 against max|expected|):
  - k-tiles 0..A_RAW-1 of the contraction run as raw fp8(e4m3) DoubleRow
    matmuls: each instruction contracts TWO 128-row k-tiles in the ~216 ns a
    normal matmul takes (2x k-throughput). Host-measured noise on the
    worst-scale input realization: ~1.5e-2 max-rel at A_RAW=5.
  - k-tiles A_RAW..31 run in bf16 (216 ns/instr, noise ~2.5e-3).
  - the whole LoRA path is fp8: u = A_all @ x via DoubleRow pairs; uTm (the
    one-hot-masked u) is written back as fp8 by the DVE; the per-block delta
    rides in the SECOND SLOT of the last DoubleRow pair (stationary slot0 =
    W^T k-tile 4, slot1 = B^C; moving slot0 = x8 k-tile 4, slot1 = uTm8), so
    the delta costs zero extra instructions on obs 1..7.
All products carry scale LAM = 2^14 (x*16, W*1024, A*1024, u*16, B*1024; bf16
W prescaled by LAM); the PSUM->SBUF descale by 2^-14 is fused into the copy,
alternating Vector/Scalar engines so back-to-back drains overlap.

Schedule (per core): ~6 us of warmup matmuls ramp the PE clock gate while the
first DMAs land; ob0's d-loop runs bf16-first (x bf16 streams in first), then
the fp8 pairs; the u-pass follows ob0; ob0's delta is applied separately; obs
1..7 fuse the delta in the special pair. Each block's final accumulation,
descale copy and output DMA are emitted together so PSUM banks recycle without
stalling the PE at superblock boundaries.
"""

import numpy as np
import ml_dtypes

from concourse import bacc, tile, mybir
from concourse.bass_utils import run_bass_kernel_spmd
import concourse.bass_utils as _bu

# Disable S3 artifact upload in the trace path (no credentials in this container).
_bu.upload_artifacts = lambda tmpdir: "local://" + tmpdir

N_CORES = 8
T = 8192
D_IN = 4096
D_OUT = 4096
L = 8          # max adapters
R = 16         # max rank
LR = L * R     # 128 = stacked adapter dim
T_SH = T // N_CORES          # 1024 tokens per core
KT = D_IN // 128             # 32 contraction tiles
KP = KT // 2                 # 16 k-pair tiles (u-pass / fp8 layout)
OB = D_OUT // 512            # 8 output-column superblocks
NO = 4                       # 128-wide output blocks per superblock
NT = T_SH // 512             # 2 token blocks (moving dim)

A_RAW = 5                    # k-tiles done in raw fp8 (4 in full pairs + 1 in
NPW = 2                      # the special pair's slot0); NPW = full W pairs
KB = KT - A_RAW              # k-tiles done in bf16

SX = 16.0                    # fp8 scale for x
SW = 1024.0                  # fp8 scale for W
SU = 16.0                    # fp8 scale for u (uTm)
SB = 1024.0                  # fp8 scale for B^C (and A^T)
LAM = SX * SW                # 2^14: scale carried by every PSUM product
ILAM = 1.0 / LAM
MASK_V = SU / LAM            # 2^-10: folded into the one-hot mask

F32 = mybir.dt.float32
F8 = mybir.dt.float8e4
BF16 = mybir.dt.bfloat16
DR = mybir.MatmulPerfMode.DoubleRow

N_WARM = 24                  # PE clock-gate warmup matmuls

_CACHED_NC = None


def _build():
    nc = bacc.Bacc("TRN2", target_bir_lowering=False, debug=False)

    xb_d = nc.dram_tensor("xb", [KB * 128, T_SH], BF16, kind="ExternalInput")
    x8u_d = nc.dram_tensor("x8u", [KP * 128, 2 * T_SH], F8, kind="ExternalInput")
    x8sp_d = nc.dram_tensor("x8sp", [128, T_SH], F8, kind="ExternalInput")
    w8_d = nc.dram_tensor("w8", [NPW * 128, 2 * D_OUT], F8, kind="ExternalInput")
    w8bc_d = nc.dram_tensor("w8bc", [128, 2 * D_OUT], F8, kind="ExternalInput")
    a8_d = nc.dram_tensor("a8", [KP * 128, 2 * LR], F8, kind="ExternalInput")
    wb_d = nc.dram_tensor("wb", [KB * 128, D_OUT], BF16, kind="ExternalInput")
    mT_d = nc.dram_tensor("maskT", [LR, T_SH], BF16, kind="ExternalInput")
    yT_d = nc.dram_tensor("yT", [D_OUT, T_SH], F32, kind="ExternalOutput")

    n_copies = [0]

    with tile.TileContext(nc) as tc:
        with (
            tc.tile_pool(name="resident", bufs=1) as rpool,
            tc.tile_pool(name="wstream", bufs=6) as w8pool,
            tc.tile_pool(name="wbcstream", bufs=4) as wbcpool,
            tc.tile_pool(name="wbstream", bufs=28) as wbpool,
            tc.tile_pool(name="yout", bufs=4) as ypool,
            tc.tile_pool(name="psum", bufs=8, space="PSUM") as psum,
        ):
            # --- resident loads in consumption order. First xb tiles split in
            # --- 4 chunks so the PE's first real matmul isn't gated on one
            # --- long single-queue transfer.
            xbs = [None] * KT
            wbs0 = []
            for db in range(KB):
                d = A_RAW + db
                xbt = rpool.tile([128, T_SH], BF16, tag=f"xb{d}", name=f"xb{d}")
                if db < 4:
                    for q in range(4):
                        csl = slice(q * (T_SH // 4), (q + 1) * (T_SH // 4))
                        nc.sync.dma_start(xbt[:, csl],
                                          xb_d[db * 128:(db + 1) * 128, csl])
                else:
                    nc.sync.dma_start(xbt[:], xb_d[db * 128:(db + 1) * 128, :])
                xbs[d] = xbt
                wbt = wbpool.tile([128, 512], BF16, tag="wb", name=f"wb0_{db}")
                nc.sync.dma_start(wbt[:], wb_d[db * 128:(db + 1) * 128, 0:512])
                wbs0.append(wbt)
            x8us = []
            for pr in range(KP):
                x8t = rpool.tile([128, 2, T_SH], F8, tag=f"x8u_{pr}",
                                 name=f"x8u_{pr}")
                nc.sync.dma_start(
                    x8t[:],
                    x8u_d[pr * 128:(pr + 1) * 128, :]
                    .rearrange("p (two t) -> p two t", two=2))
                x8us.append(x8t)
                if pr == 1:
                    # special pair slot0 (k-tile 4) + ob0's W fp8 right after
                    # the two base pairs so ob0's fp8 phase isn't starved
                    x8sp = rpool.tile([128, 2, T_SH], F8, tag="x8sp")
                    nc.sync.dma_start(x8sp[:, 0, :], x8sp_d[:])
                    w8s0 = []
                    for pw_ in range(NPW):
                        w8t = w8pool.tile([128, 2, 512], F8, tag="w8",
                                          name=f"w80_{pw_}")
                        nc.sync.dma_start(
                            w8t[:],
                            w8_d[pw_ * 128:(pw_ + 1) * 128, :]
                            .rearrange("p (two o) -> p two o", two=2)[:, :, 0:512])
                        w8s0.append(w8t)
                    # resident (not the streaming ring): its slot1 (bc) is
                    # read again by the ob0-delta phase at the very end
                    wbc0 = rpool.tile([128, 2, 512], F8, tag="wbc0")
                    nc.sync.dma_start(
                        wbc0[:],
                        w8bc_d[:].rearrange("p (two o) -> p two o", two=2)
                        [:, :, 0:512])
            a8s = []
            for pr in range(KP):
                a8t = rpool.tile([128, 2, LR], F8, tag=f"a8_{pr}", name=f"a8_{pr}")
                nc.sync.dma_start(
                    a8t[:],
                    a8_d[pr * 128:(pr + 1) * 128, :]
                    .rearrange("p (two t) -> p two t", two=2))
                a8s.append(a8t)
            mask = rpool.tile([LR, T_SH], BF16, tag="mask")
            nc.sync.dma_start(mask[:], mT_d[:])

            def copy_descale(dst, src):
                if n_copies[0] % 2 == 0:
                    nc.vector.tensor_scalar_mul(dst, src, ILAM)
                else:
                    nc.scalar.activation(
                        dst, src, mybir.ActivationFunctionType.Copy, scale=ILAM)
                n_copies[0] += 1

            def base_accum(pys, w8t, wbc, wbts, finish, fuse_delta):
                """One 512-col superblock into pys[o][t]: bf16 k-tiles, two
                full fp8 pairs, then the special pair (k-tile 4 + fused delta
                when fuse_delta). finish(o, t) is emitted right after each
                block's last accumulation so the PSUM drain overlaps the PE."""
                for db in range(KB):
                    d = A_RAW + db
                    for o in range(NO):
                        lw = wbts[db][:, o * 128:(o + 1) * 128]
                        for t in range(NT):
                            nc.tensor.matmul(
                                pys[o][t][:], lw,
                                xbs[d][:, t * 512:(t + 1) * 512],
                                start=(db == 0), stop=False,
                                skip_group_check=True)
                for pw_ in range(NPW):
                    for o in range(NO):
                        lw = w8t[pw_][:, :, o * 128:(o + 1) * 128]
                        for t in range(NT):
                            nc.tensor.matmul(
                                pys[o][t][:], lw,
                                x8us[pw_][:, :, t * 512:(t + 1) * 512],
                                start=False, stop=False,
                                perf_mode=DR, skip_group_check=True)
                for o in range(NO):
                    for t in range(NT):
                        if fuse_delta:
                            nc.tensor.matmul(
                                pys[o][t][:],
                                wbc[:, :, o * 128:(o + 1) * 128],
                                x8sp[:, :, t * 512:(t + 1) * 512],
                                start=False, stop=True,
                                perf_mode=DR, skip_group_check=True)
                        else:
                            nc.tensor.matmul(
                                pys[o][t][:],
                                wbc[:, 0, o * 128:(o + 1) * 128],
                                x8sp[:, 0, t * 512:(t + 1) * 512],
                                start=False, stop=True,
                                skip_group_check=True)
                        finish(o, t)

            # --- phase 1: ob0 d-loop (base matmul only; delta comes later) -----
            pys0 = [[psum.tile([128, 512], F32, tag="acc", name=f"py0_{o}_{t}")
                     for t in range(NT)] for o in range(NO)]
            yo0s = {}

            def finish0(o, t):
                yo0 = rpool.tile([128, 512], F32, tag=f"yo0_{o}_{t}",
                                 name=f"yo0_{o}_{t}")
                copy_descale(yo0[:], pys0[o][t][:])
                yo0s[o, t] = yo0

            base_accum(pys0, w8s0, wbc0, wbs0, finish0, fuse_delta=False)

            # --- phase 2: u-pass, all fp8 DoubleRow; uTm8 lands in x8sp slot1 --
            for ub in range(NT):
                pu = psum.tile([128, 512], F32, tag="acc", name=f"pu{ub}")
                sl = slice(ub * 512, (ub + 1) * 512)
                for pr in range(KP):
                    nc.tensor.matmul(
                        pu[:], a8s[pr][:], x8us[pr][:, :, sl],
                        start=(pr == 0), stop=(pr == KP - 1),
                        perf_mode=DR, skip_group_check=True)
                # mask = one-hot * 2^-10: pu (= 2^14 u) -> uTm8 = fp8(16 u)
                nc.vector.tensor_mul(x8sp[:, 1, sl], pu[:], mask[:, sl])

            # --- phase 3: ob1..7 with the delta fused in the special pair ------
            for ob in range(1, OB):
                osl = slice(ob * 512, (ob + 1) * 512)
                w8t = []
                for pw_ in range(NPW):
                    w8x = w8pool.tile([128, 2, 512], F8, tag="w8",
                                      name=f"w8_{ob}_{pw_}")
                    nc.sync.dma_start(
                        w8x[:],
                        w8_d[pw_ * 128:(pw_ + 1) * 128, :]
                        .rearrange("p (two o) -> p two o", two=2)[:, :, osl])
                    w8t.append(w8x)
                wbc = wbcpool.tile([128, 2, 512], F8, tag="wbc", name=f"wbc{ob}")
                nc.sync.dma_start(
                    wbc[:],
                    w8bc_d[:].rearrange("p (two o) -> p two o", two=2)[:, :, osl])
                wbts = []
                for db in range(KB):
                    wbt = wbpool.tile([128, 512], BF16, tag="wb",
                                      name=f"wb{ob}_{db}")
                    nc.sync.dma_start(wbt[:], wb_d[db * 128:(db + 1) * 128, osl])
                    wbts.append(wbt)
                pys = [[psum.tile([128, 512], F32, tag="acc", name=f"py{ob}_{o}_{t}")
                        for t in range(NT)] for o in range(NO)]

                def finish(o, t, ob=ob, pys=pys):
                    og = ob * 512 + o * 128
                    yo = ypool.tile([128, 512], F32, tag="yo",
                                    name=f"yo{ob}_{o}_{t}")
                    copy_descale(yo[:], pys[o][t][:])
                    nc.sync.dma_start(
                        yT_d[og:og + 128, t * 512:(t + 1) * 512], yo[:])

                base_accum(pys, w8t, wbc, wbts, finish, fuse_delta=True)

            # --- phase 4: ob0 delta + writeback (fills the tail drain) ---------
            for o in range(NO):
                for t in range(NT):
                    pd = psum.tile([128, 512], F32, tag="acc", name=f"pd{o}_{t}")
                    nc.tensor.matmul(
                        pd[:], wbc0[:, 1, o * 128:(o + 1) * 128],
                        x8sp[:, 1, t * 512:(t + 1) * 512],
                        start=True, stop=True, skip_group_check=True)
                    yo = ypool.tile([128, 512], F32, tag="yo", name=f"yod{o}_{t}")
                    # yo = pd*2^-14 + yo0 (yo0 already descaled)
                    nc.vector.scalar_tensor_tensor(
                        yo[:], pd[:], ILAM, yo0s[o, t][:],
                        mybir.AluOpType.mult, mybir.AluOpType.add)
                    nc.sync.dma_start(
                        yT_d[o * 128:(o + 1) * 128, t * 512:(t + 1) * 512], yo[:])

    nc.compile()
    return nc


def _get_nc():
    global _CACHED_NC
    if _CACHED_NC is None:
        _CACHED_NC = _build()
    return _CACHED_NC


def _q8(v, s):
    return np.clip(v * s, -240.0, 240.0).astype(ml_dtypes.float8_e4m3fn)


def _pairs(arr, width):
    """[2n*128, width] -> pair-major rows [n*128, 2*width]."""
    n = arr.shape[0] // 256
    return np.ascontiguousarray(
        arr.reshape(n, 2, 128, width).transpose(0, 2, 1, 3)
           .reshape(n * 128, 2 * width))


def _prep_in_maps(x, weight, lora_A, lora_B, token_to_slot):
    x = np.asarray(x, dtype=np.float32)
    weight = np.asarray(weight, dtype=np.float32)
    lora_A = np.asarray(lora_A, dtype=np.float32)
    lora_B = np.asarray(lora_B, dtype=np.float32)
    slots = np.asarray(token_to_slot)

    wT = weight.T                                                         # [D_IN, D_OUT]
    aT = lora_A.transpose(2, 0, 1).reshape(D_IN, LR)                      # [D_IN, L*R]
    bc = lora_B.transpose(0, 2, 1).reshape(LR, D_OUT)                     # [L*R, D_OUT]

    w8 = _pairs(_q8(wT[:NPW * 256, :], SW), D_OUT)
    w8bc = np.ascontiguousarray(
        np.concatenate([_q8(wT[4 * 128:5 * 128, :], SW),
                        _q8(bc, SB)], axis=1))                            # [128, 2*D_OUT]
    a8 = _pairs(_q8(aT, SB), LR)
    wb = np.ascontiguousarray((wT[A_RAW * 128:, :] * LAM).astype(ml_dtypes.bfloat16))

    # One-hot mask over stacked adapter rows, with 2^-10 folded in (descale by
    # 2^-14 plus the uTm fp8 scale 16); out-of-range slots -> all-zero.
    maskT = np.zeros((LR, T), dtype=np.float32)
    for l in range(L):
        maskT[l * R:(l + 1) * R, :] = (slots == l).astype(np.float32)[None, :] * MASK_V

    in_maps = []
    for c in range(N_CORES):
        tsl = slice(c * T_SH, (c + 1) * T_SH)
        xT = x[tsl, :].T                                                  # [D_IN, T_SH]
        x8full = _q8(xT, SX)
        in_maps.append({
            "xb": np.ascontiguousarray(xT[A_RAW * 128:, :].astype(ml_dtypes.bfloat16)),
            "x8u": _pairs(x8full, T_SH),
            "x8sp": np.ascontiguousarray(x8full[4 * 128:5 * 128, :]),
            "w8": w8,
            "w8bc": w8bc,
            "a8": a8,
            "wb": wb,
            "maskT": np.ascontiguousarray(maskT[:, tsl]).astype(ml_dtypes.bfloat16),
        })
    return in_maps


def _run(inputs, trace=False, trace_cores=None):
    nc = _get_nc()
    in_maps = _prep_in_maps(**inputs)
    res = run_bass_kernel_spmd(
        nc, in_maps, core_ids=list(range(N_CORES)),
        trace=trace, trace_cores=trace_cores,
    )
    y = np.concatenate([res.results[c]["yT"].T for c in range(N_CORES)], axis=0)
    y = np.ascontiguousarray(y)
    return y, res


def _bf(v):
    return np.asarray(v, np.float32).astype(ml_dtypes.bfloat16).astype(np.float64)


def _validate(inputs, y):
    """Cheap host-side sanity check: project y onto a random vector and compare
    with a QUANTIZATION-AWARE host projection (so fp8/bf16 noise cancels and
    the tolerance can stay tight). Catches transient device corruption; costs
    well under 1 s on host BLAS (only matvecs against r)."""
    if y is None or not np.all(np.isfinite(y)):
        return False
    x = np.asarray(inputs["x"], dtype=np.float32)
    weight = np.asarray(inputs["weight"], dtype=np.float32)
    lora_A = np.asarray(inputs["lora_A"], dtype=np.float32)
    lora_B = np.asarray(inputs["lora_B"], dtype=np.float32)
    slots = np.asarray(inputs["token_to_slot"])

    rng = np.random.default_rng(12345)
    r = rng.standard_normal(D_OUT).astype(np.float64)

    wT = weight.T                                                         # [D_IN, D_OUT]
    xT = x.T                                                              # [D_IN, T]
    AR = A_RAW * 128
    # raw fp8 part (scaled values; descale at the end)
    w8r = _q8(wT[:AR, :], SW).astype(np.float64) @ r                      # [AR]
    x8 = _q8(xT, SX).astype(np.float64)                                   # [D_IN, T]
    p_raw = (x8[:AR, :].T @ w8r) * ILAM
    # bf16 part
    wbr = _bf(wT[AR:, :] * LAM) @ r
    p_bf = (_bf(xT[AR:, :]).T @ wbr) * ILAM
    # fp8 LoRA path exactly as on device
    aT = lora_A.transpose(2, 0, 1).reshape(D_IN, LR)
    a8 = _q8(aT, SB).astype(np.float64)                                   # [D_IN, LR]
    u_q = (a8.T @ x8) * ILAM                                              # logical u
    m = np.zeros((LR, T))
    for l in range(L):
        m[l * R:(l + 1) * R, :] = (slots == l).astype(np.float64)[None, :]
    uTm8 = _q8((u_q * SU).astype(np.float32), 1.0).astype(np.float64) * m  # scaled SU
    bc8r = _q8(lora_B.transpose(0, 2, 1).reshape(LR, D_OUT), SB).astype(np.float64) @ r
    p_delta = (uTm8.T @ bc8r) * ILAM

    exp = p_raw + p_bf + p_delta
    got = y.astype(np.float64) @ r
    scale = np.abs(exp).max()
    rel = np.abs(got - exp).max() / scale
    return rel < 2e-3


def kernel(x, weight, lora_A, lora_B, token_to_slot):
    inputs = dict(x=x, weight=weight, lora_A=lora_A, lora_B=lora_B,
                  token_to_slot=token_to_slot)
    y = None
    for _attempt in range(3):
        y, _ = _run(inputs)
        if _validate(inputs, y):
            break
    return y
